# revision 10
# baseline (speedup 1.0000x reference)
"""Trainium2 Bass kernel for nn_Attention (non-local-attention block + sync BN).

Computation per batch element b (B=8, C_IN=256, C_OUT=128, N=4096):
    theta = theta_w @ x + theta_b          [128, 4096]
    phi   = phi_w @ x + phi_b              [128, 4096]
    g     = g_w @ x + g_b                  [128, 4096]
    f     = theta^T @ phi / N              [4096, 4096]   (never materialized in DRAM)
    y     = g @ f^T                        [128, 4096]
    w_y   = W_w @ y  (+ W_b, cancels in BN)[256, 4096]
    out   = BN(w_y) * gamma + beta + x     (BN stats over all (B, N) -> AllReduce)

Sharding: data-parallel over batch across 8 NeuronCores (one element per
core); 1x1-conv weights replicated; BN batch stats synced with a tiny
[128,4] fp32 AllReduce.  Compute dtype bf16 (fp32 PSUM accumulation).

Main loop is software-pipelined: the y-matmul for fT pair i is emitted
LAG iterations after the fT matmuls of pair i, so the PSUM->SBUF copy of
fT (split between the Vector and Scalar engines) overlaps with later fT
matmuls and the PE stream stays dense (keeps the HAM clock gate at 2.4GHz).
"""

import contextlib

import numpy as np
import ml_dtypes

import concourse.bass as bass  # noqa: F401  (registers engines)
import concourse.tile as tile
from concourse import bacc, mybir
from concourse import bass_utils

N_CORES = 8
B, C_IN, C_OUT, N = 8, 256, 128, 4096
P = 128
NCH = N // 512    # 8 column chunks of 512
MCH = N // 128    # 32 m-chunks of 128
KPAIR = MCH // 2  # 16 fT pairs per n-chunk
LAG = 3           # y-matmul lag (iterations) behind fT matmuls
BN_EPS = 1e-5

F32 = mybir.dt.float32
BF16 = mybir.dt.bfloat16
AF = mybir.ActivationFunctionType
ALU = mybir.AluOpType
AX = mybir.AxisListType


def _build_module():
    nc = bacc.Bacc("TRN2", target_bir_lowering=False, debug=False,
                   enable_asserts=True, num_devices=N_CORES)

    x32 = nc.dram_tensor("x32", [C_IN, N], F32, kind="ExternalInput").ap()
    x16 = nc.dram_tensor("x16", [C_IN, N], BF16, kind="ExternalInput").ap()
    # wpack columns: thw0 thw1 phw0 phw1 gw0 gw1 WwA WwB (8 x [128,128] bf16)
    wpack = nc.dram_tensor("wpack", [P, 1024], BF16, kind="ExternalInput").ap()
    # bpack columns: thb(1) phb(1) gam(2) bet(2) gbb(128)
    bpack = nc.dram_tensor("bpack", [P, 134], F32, kind="ExternalInput").ap()
    out = nc.dram_tensor("out", [C_IN, N], F32, kind="ExternalOutput").ap()

    with contextlib.ExitStack() as ctx:
        tc = ctx.enter_context(tile.TileContext(nc))
        pp = ctx.enter_context(tc.tile_pool(name="persist", bufs=1))
        ftsb = ctx.enter_context(tc.tile_pool(name="ftsb", bufs=5))
        ysb = ctx.enter_context(tc.tile_pool(name="ysb", bufs=2))
        sqp = ctx.enter_context(tc.tile_pool(name="sqp", bufs=2))
        op = ctx.enter_context(tc.tile_pool(name="outp", bufs=6))
        ps_cv = ctx.enter_context(tc.tile_pool(name="pscv", bufs=2, space="PSUM"))
        ps_ft = ctx.enter_context(tc.tile_pool(name="psft", bufs=2, space="PSUM"))
        ps_y = ctx.enter_context(tc.tile_pool(name="psy", bufs=2, space="PSUM"))
        dram = ctx.enter_context(tc.tile_pool(name="dram", bufs=1, space="DRAM"))

        # ---- persistent SBUF tensors ----
        x16h = [pp.tile([P, N], BF16, tag=f"x16_{h}", name=f"x16_{h}")
                for h in range(2)]
        x32h = [pp.tile([P, N], F32, tag=f"x32_{h}", name=f"x32_{h}")
                for h in range(2)]
        th_t = pp.tile([P, N], BF16, tag="th")
        ph_t = pp.tile([P, N], BF16, tag="ph")
        gt_t = pp.tile([P, N], BF16, tag="gt")       # g^T in 32 [128m x 128c] blocks
        wy_t = [pp.tile([P, N], F32, tag=f"wy{h}", name=f"wy{h}") for h in range(2)]
        stat_s = pp.tile([P, 16], F32, tag="stat_s")  # per-chunk sums
        stat_q = pp.tile([P, 16], F32, tag="stat_q")  # per-chunk sum-of-squares

        wp_t = pp.tile([P, 1024], BF16, tag="wp")
        bp_t = pp.tile([P, 134], F32, tag="bp")
        eps_t = pp.tile([P, 1], F32, tag="eps")
        nc.gpsimd.memset(eps_t[:], BN_EPS)
        warm_t = pp.tile([P, 1], F32, tag="warm")
        nc.scalar.activation(warm_t[:], eps_t[:], AF.Sqrt)  # preload ACT table

        def cs(i, w):  # column slice helper
            return slice(i * w, (i + 1) * w)

        # weight DMAs first (small), then x16 chunks so the convs start early,
        # x32 last via SWDGE (only needed for the tail residual)
        nc.sync.dma_start(wp_t[:], wpack[:, :])
        nc.sync.dma_start(bp_t[:], bpack[:, :])
        for j in range(NCH):
            nc.sync.dma_start(x16h[0][:, cs(j, 512)], x16[0:P, cs(j, 512)])
            nc.scalar.dma_start(x16h[1][:, cs(j, 512)], x16[P:2 * P, cs(j, 512)])
        for h in range(2):
            nc.gpsimd.dma_start(x32h[h][:], x32[h * P:(h + 1) * P, :])

        thw_t = [wp_t[:, cs(k, P)] for k in range(2)]
        phw_t = [wp_t[:, cs(2 + k, P)] for k in range(2)]
        gw_t = [wp_t[:, cs(4 + k, P)] for k in range(2)]
        Ww_h = [wp_t[:, cs(6 + h, P)] for h in range(2)]
        thb_t = bp_t[:, 0:1]
        phb_t = bp_t[:, 1:2]
        gam_t = bp_t[:, 2:4]
        bet_t = bp_t[:, 4:6]
        gbb_t = bp_t[:, 6:134]

        # ---- phi / theta convs, interleaved per chunk (DMA-paced) ----
        for j in range(NCH):
            for (w_t, b_t, dst) in ((phw_t, phb_t, ph_t), (thw_t, thb_t, th_t)):
                ps = ps_cv.tile([P, 512], F32, tag="cv", name="ps_conv")
                nc.tensor.matmul(ps[:], w_t[0], x16h[0][:, cs(j, 512)],
                                 start=True, stop=False)
                nc.tensor.matmul(ps[:], w_t[1], x16h[1][:, cs(j, 512)],
                                 start=False, stop=True)
                nc.scalar.activation(dst[:, cs(j, 512)], ps[:], AF.Identity,
                                     bias=b_t)

        def emit_gt_conv(m):
            ps = ps_cv.tile([P, P], F32, tag="cv", name="ps_gt")
            nc.tensor.matmul(ps[:], x16h[0][:, cs(m, P)], gw_t[0],
                             start=True, stop=False)
            nc.tensor.matmul(ps[:], x16h[1][:, cs(m, P)], gw_t[1],
                             start=False, stop=True)
            nc.vector.tensor_tensor(gt_t[:, cs(m, P)], ps[:], gbb_t[:],
                                    op=ALU.add)

        def emit_w_block(j, y_sb):
            for h in range(2):
                w_ps = ps_cv.tile([P, 512], F32, tag="cv", name="ps_w")
                nc.tensor.matmul(w_ps[:], Ww_h[h], y_sb[:],
                                 start=True, stop=True)
                nc.scalar.activation(wy_t[h][:, cs(j, 512)], w_ps[:], AF.Copy)
                col = h * NCH + j
                wyc = wy_t[h][:, cs(j, 512)]
                sc = sqp.tile([P, 512], F32, tag="sc", name="sc")
                nc.vector.tensor_scalar(sc[:], wyc, 1.0, 0.0, op0=ALU.mult,
                                        op1=ALU.add,
                                        accum_out=stat_s[:, col:col + 1])
                sq = sqp.tile([P, 512], F32, tag="sq", name="sq")
                nc.scalar.activation(sq[:], wyc, AF.Square,
                                     accum_out=stat_q[:, col:col + 1])

        # ---- software-pipelined main loop over flattened (j, k) pairs ----
        TOT = NCH * KPAIR  # 128
        ft_sbs = {}
        y_ps_cur = [None]
        pending_w = []  # (emit_at_iter, j, y_sb)

        for it in range(TOT + LAG):
            # gT convs embedded into the first iterations (2 per iter)
            if it < MCH // 2:
                emit_gt_conv(2 * it)
                emit_gt_conv(2 * it + 1)

            if it < TOT:
                j, k = divmod(it, KPAIR)
                ft_ps = ps_ft.tile([P, 1024], F32, tag="ft", name="ft_ps")
                nc.tensor.matmul(ft_ps[:, 0:512], ph_t[:, cs(2 * k, P)],
                                 th_t[:, cs(j, 512)], start=True, stop=True)
                nc.tensor.matmul(ft_ps[:, 512:1024], ph_t[:, cs(2 * k + 1, P)],
                                 th_t[:, cs(j, 512)], start=True, stop=True)
                ft_sb = ftsb.tile([P, 1024], BF16, tag="ft_sb", name="ft_sb")
                if it % 2 == 0:
                    nc.vector.tensor_copy(ft_sb[:], ft_ps[:])
                else:
                    nc.scalar.activation(ft_sb[:], ft_ps[:], AF.Copy)
                ft_sbs[it] = ft_sb

            while pending_w and pending_w[0][0] <= it:
                _, jw, y_sb_w = pending_w.pop(0)
                emit_w_block(jw, y_sb_w)

            iy = it - LAG
            if 0 <= iy < TOT:
                j2, k2 = divmod(iy, KPAIR)
                if k2 == 0:
                    y_ps_cur[0] = ps_y.tile([P, 512], F32, tag="y", name="y_ps")
                y_ps = y_ps_cur[0]
                ft_sb = ft_sbs.pop(iy)
                nc.tensor.matmul(y_ps[:], gt_t[:, cs(2 * k2, P)],
                                 ft_sb[:, 0:512], start=(k2 == 0), stop=False)
                nc.tensor.matmul(y_ps[:], gt_t[:, cs(2 * k2 + 1, P)],
                                 ft_sb[:, 512:1024], start=False,
                                 stop=(k2 == KPAIR - 1))
                if k2 == KPAIR - 1:
                    y_sb = ysb.tile([P, 512], BF16, tag="y_sb", name="y_sb")
                    nc.vector.tensor_copy(y_sb[:], y_ps[:])
                    pending_w.append((it + 2, j2, y_sb))

        while pending_w:
            _, jw, y_sb_w = pending_w.pop(0)
            emit_w_block(jw, y_sb_w)

        # ---- BN stats: local reduce, AllReduce, affine params ----
        s4 = pp.tile([P, 4], F32, tag="s4")
        nc.vector.reduce_sum(s4[:, 0:1], stat_s[:, 0:NCH], axis=AX.X)
        nc.vector.reduce_sum(s4[:, 1:2], stat_s[:, NCH:2 * NCH], axis=AX.X)
        nc.vector.reduce_sum(s4[:, 2:3], stat_q[:, 0:NCH], axis=AX.X)
        nc.vector.reduce_sum(s4[:, 3:4], stat_q[:, NCH:2 * NCH], axis=AX.X)
        del stat_s, stat_q
        in_b = dram.tile([P, 4], F32)
        out_b = dram.tile([P * N_CORES, 4], F32)
        nc.sync.dma_start(in_b[:], s4[:])
        nc.gpsimd.collective_compute(
            "AllGather", ALU.bypass,
            replica_groups=[list(range(N_CORES))],
            ins=[in_b.opt()], outs=[out_b.opt()],
        )
        g32 = pp.tile([P, 32], F32, tag="g32")
        nc.sync.dma_start(g32[:].rearrange("p (c r) -> p c r", r=N_CORES),
                          out_b[:].rearrange("(r p) c -> p c r", p=P))
        g4 = pp.tile([P, 4], F32, tag="g4")
        nc.vector.reduce_sum(g4[:], g32[:].rearrange("p (c r) -> p c r",
                                                     r=N_CORES), axis=AX.X)

        inv_cnt = 1.0 / (B * N)
        m4 = pp.tile([P, 4], F32, tag="m4")      # [mn(2) | ms(2)]
        var = pp.tile([P, 2], F32, tag="var")
        tmp = pp.tile([P, 2], F32, tag="tmp")
        sd = pp.tile([P, 2], F32, tag="sd")
        rstd = pp.tile([P, 2], F32, tag="rstd")
        scl = pp.tile([P, 2], F32, tag="scl")
        bia = pp.tile([P, 2], F32, tag="bia")
        nc.vector.tensor_scalar_mul(m4[:], g4[:], inv_cnt)
        mn = m4[:, 0:2]
        nc.vector.tensor_mul(tmp[:], mn, mn)
        nc.vector.tensor_sub(var[:], m4[:, 2:4], tmp[:])
        nc.scalar.activation(sd[:], var[:], AF.Sqrt, bias=eps_t[:, 0:1])
        nc.vector.reciprocal(rstd[:], sd[:])
        nc.vector.tensor_mul(scl[:], rstd[:], gam_t)
        nc.vector.tensor_mul(tmp[:], mn, scl[:])
        nc.vector.tensor_sub(bia[:], bet_t, tmp[:])

        # ---- normalize + residual + store ----
        for idx in range(2 * NCH):
            h, j = divmod(idx, NCH)
            o1 = op.tile([P, 512], F32, tag="o1", name="o1")
            nc.scalar.activation(o1[:], wy_t[h][:, cs(j, 512)], AF.Identity,
                                 bias=bia[:, h:h + 1], scale=scl[:, h:h + 1])
            o2 = op.tile([P, 512], F32, tag="o2", name="o2")
            nc.vector.tensor_add(o2[:], o1[:], x32h[h][:, cs(j, 512)])
            nc.sync.dma_start(out[h * P:(h + 1) * P, cs(j, 512)], o2[:])

    nc.compile()
    return nc


_CACHE = {}


def _get_module():
    if "nc" not in _CACHE:
        _CACHE["nc"] = _build_module()
    return _CACHE["nc"]


def _prep_in_maps(x, g_w, g_b, theta_w, theta_b, phi_w, phi_b, W_w, W_b,
                  bn_gamma, bn_beta):
    bf = ml_dtypes.bfloat16
    f32 = np.float32
    x = np.ascontiguousarray(x, dtype=f32)
    thwT = (theta_w.T / N).astype(bf)
    phwT = phi_w.T.astype(bf)
    gwT = g_w.T.astype(bf)
    WwT = W_w.T.astype(bf)
    wpack = np.concatenate(
        [thwT[0:P], thwT[P:2 * P], phwT[0:P], phwT[P:2 * P],
         gwT[0:P], gwT[P:2 * P], WwT[:, 0:P], WwT[:, P:2 * P]], axis=1)
    bpack = np.concatenate(
        [(theta_b / N).reshape(P, 1).astype(f32),
         phi_b.reshape(P, 1).astype(f32),
         bn_gamma.reshape(2, P).T.astype(f32),
         bn_beta.reshape(2, P).T.astype(f32),
         np.broadcast_to(g_b[None, :].astype(f32), (P, C_OUT))], axis=1)
    shared = {
        "wpack": np.ascontiguousarray(wpack),
        "bpack": np.ascontiguousarray(bpack),
    }
    in_maps = []
    for i in range(N_CORES):
        m = dict(shared)
        m["x32"] = x[i]
        m["x16"] = np.ascontiguousarray(x[i].astype(bf))
        in_maps.append(m)
    return in_maps


def _run(inputs, trace=False, trace_cores=None):
    nc = _get_module()
    in_maps = _prep_in_maps(**inputs)
    res = bass_utils.run_bass_kernel_spmd(
        nc, in_maps, core_ids=list(range(N_CORES)),
        trace=trace, trace_cores=trace_cores,
    )
    out = np.stack([res.results[i]["out"] for i in range(N_CORES)], axis=0)
    return out.astype(np.float32), res


def kernel(**inputs) -> np.ndarray:
    out, _ = _run(inputs, trace=False)
    return out


# revision 11
# speedup vs baseline: 1.0117x; 1.0117x over previous
"""Trainium2 Bass kernel for nn_Attention (non-local-attention block + sync BN).

Computation per batch element b (B=8, C_IN=256, C_OUT=128, N=4096):
    theta = theta_w @ x + theta_b          [128, 4096]
    phi   = phi_w @ x + phi_b              [128, 4096]
    g     = g_w @ x + g_b                  [128, 4096]
    f     = theta^T @ phi / N              [4096, 4096]   (never materialized in DRAM)
    y     = g @ f^T                        [128, 4096]
    w_y   = W_w @ y  (+ W_b, cancels in BN)[256, 4096]
    out   = BN(w_y) * gamma + beta + x     (BN stats over all (B, N) -> AllReduce)

Sharding: data-parallel over batch across 8 NeuronCores (one element per
core); 1x1-conv weights replicated; BN batch stats synced with a tiny
[128,4] fp32 AllReduce.  Compute dtype bf16 (fp32 PSUM accumulation).

Main loop is software-pipelined: the y-matmul for fT pair i is emitted
LAG iterations after the fT matmuls of pair i, so the PSUM->SBUF copy of
fT (split between the Vector and Scalar engines) overlaps with later fT
matmuls and the PE stream stays dense (keeps the HAM clock gate at 2.4GHz).
"""

import contextlib

import numpy as np
import ml_dtypes

import concourse.bass as bass  # noqa: F401  (registers engines)
import concourse.tile as tile
from concourse import bacc, mybir
from concourse import bass_utils

N_CORES = 8
B, C_IN, C_OUT, N = 8, 256, 128, 4096
P = 128
NCH = N // 512    # 8 column chunks of 512
MCH = N // 128    # 32 m-chunks of 128
KPAIR = MCH // 2  # 16 fT pairs per n-chunk
LAG = 3           # y-matmul lag (iterations) behind fT matmuls
BN_EPS = 1e-5

F32 = mybir.dt.float32
BF16 = mybir.dt.bfloat16
AF = mybir.ActivationFunctionType
ALU = mybir.AluOpType
AX = mybir.AxisListType


def _build_module():
    nc = bacc.Bacc("TRN2", target_bir_lowering=False, debug=False,
                   enable_asserts=True, num_devices=N_CORES)

    x32 = nc.dram_tensor("x32", [C_IN, N], F32, kind="ExternalInput").ap()
    x16 = nc.dram_tensor("x16", [C_IN, N], BF16, kind="ExternalInput").ap()
    # wpack columns: thw0 thw1 phw0 phw1 gw0 gw1 WwA WwB (8 x [128,128] bf16)
    wpack = nc.dram_tensor("wpack", [P, 1024], BF16, kind="ExternalInput").ap()
    # bpack columns: thb(1) phb(1) gam(2) bet(2) gbb(128)
    bpack = nc.dram_tensor("bpack", [P, 134], F32, kind="ExternalInput").ap()
    out = nc.dram_tensor("out", [C_IN, N], F32, kind="ExternalOutput").ap()

    with contextlib.ExitStack() as ctx:
        tc = ctx.enter_context(tile.TileContext(nc))
        pp = ctx.enter_context(tc.tile_pool(name="persist", bufs=1))
        ftsb = ctx.enter_context(tc.tile_pool(name="ftsb", bufs=5))
        ysb = ctx.enter_context(tc.tile_pool(name="ysb", bufs=2))
        sqp = ctx.enter_context(tc.tile_pool(name="sqp", bufs=2))
        op = ctx.enter_context(tc.tile_pool(name="outp", bufs=6))
        ps_cv = ctx.enter_context(tc.tile_pool(name="pscv", bufs=2, space="PSUM"))
        ps_ft = ctx.enter_context(tc.tile_pool(name="psft", bufs=2, space="PSUM"))
        ps_y = ctx.enter_context(tc.tile_pool(name="psy", bufs=2, space="PSUM"))
        dram = ctx.enter_context(tc.tile_pool(name="dram", bufs=1, space="DRAM"))

        # ---- persistent SBUF tensors ----
        x16h = [pp.tile([P, N], BF16, tag=f"x16_{h}", name=f"x16_{h}")
                for h in range(2)]
        x32h = [pp.tile([P, N], F32, tag=f"x32_{h}", name=f"x32_{h}")
                for h in range(2)]
        th_t = pp.tile([P, N], BF16, tag="th")
        ph_t = pp.tile([P, N], BF16, tag="ph")
        gt_t = pp.tile([P, N], BF16, tag="gt")       # g^T in 32 [128m x 128c] blocks
        wy_t = [pp.tile([P, N], F32, tag=f"wy{h}", name=f"wy{h}") for h in range(2)]
        stat_s = pp.tile([P, 16], F32, tag="stat_s")  # per-chunk sums
        stat_q = pp.tile([P, 16], F32, tag="stat_q")  # per-chunk sum-of-squares

        wp_t = pp.tile([P, 1024], BF16, tag="wp")
        bp_t = pp.tile([P, 134], F32, tag="bp")
        eps_t = pp.tile([P, 1], F32, tag="eps")
        nc.gpsimd.memset(eps_t[:], BN_EPS)
        warm_t = pp.tile([P, 1], F32, tag="warm")

        def cs(i, w):  # column slice helper
            return slice(i * w, (i + 1) * w)

        # weight DMAs first (small), then x16 chunks so the convs start early,
        # x32 last via SWDGE (only needed for the tail residual)
        nc.sync.dma_start(wp_t[:], wpack[:, :])
        nc.sync.dma_start(bp_t[:], bpack[:, :])
        for j in range(NCH):
            nc.sync.dma_start(x16h[0][:, cs(j, 512)], x16[0:P, cs(j, 512)])
            nc.scalar.dma_start(x16h[1][:, cs(j, 512)], x16[P:2 * P, cs(j, 512)])
        thw_t = [wp_t[:, cs(k, P)] for k in range(2)]
        phw_t = [wp_t[:, cs(2 + k, P)] for k in range(2)]
        gw_t = [wp_t[:, cs(4 + k, P)] for k in range(2)]
        Ww_h = [wp_t[:, cs(6 + h, P)] for h in range(2)]
        thb_t = bp_t[:, 0:1]
        phb_t = bp_t[:, 1:2]
        gam_t = bp_t[:, 2:4]
        bet_t = bp_t[:, 4:6]
        gbb_t = bp_t[:, 6:134]

        # dummy tiny AllGather: warms the ncfw path early, overlapped with
        # compute, so the real stats collective at the tail runs at floor cost
        in_d = dram.tile([P, 1], F32)
        out_d = dram.tile([P * N_CORES, 1], F32)
        nc.sync.dma_start(in_d[:], eps_t[:])
        nc.gpsimd.collective_compute(
            "AllGather", ALU.bypass,
            replica_groups=[list(range(N_CORES))],
            ins=[in_d.opt()], outs=[out_d.opt()],
        )
        nc.sync.dma_start(warm_t[:], out_d[0:P, :])

        # ---- phi / theta convs, interleaved per chunk (DMA-paced) ----
        for j in range(NCH):
            for (w_t, b_t, dst) in ((phw_t, phb_t, ph_t), (thw_t, thb_t, th_t)):
                ps = ps_cv.tile([P, 512], F32, tag="cv", name="ps_conv")
                nc.tensor.matmul(ps[:], w_t[0], x16h[0][:, cs(j, 512)],
                                 start=True, stop=False)
                nc.tensor.matmul(ps[:], w_t[1], x16h[1][:, cs(j, 512)],
                                 start=False, stop=True)
                nc.scalar.activation(dst[:, cs(j, 512)], ps[:], AF.Identity,
                                     bias=b_t)

        nc.scalar.activation(warm_t[:], eps_t[:], AF.Sqrt)  # preload ACT table

        def emit_gt_conv(m):
            ps = ps_cv.tile([P, P], F32, tag="cv", name="ps_gt")
            nc.tensor.matmul(ps[:], x16h[0][:, cs(m, P)], gw_t[0],
                             start=True, stop=False)
            nc.tensor.matmul(ps[:], x16h[1][:, cs(m, P)], gw_t[1],
                             start=False, stop=True)
            nc.vector.tensor_tensor(gt_t[:, cs(m, P)], ps[:], gbb_t[:],
                                    op=ALU.add)

        def emit_w_block(j, y_sb):
            for h in range(2):
                w_ps = ps_cv.tile([P, 512], F32, tag="cv", name="ps_w")
                nc.tensor.matmul(w_ps[:], Ww_h[h], y_sb[:],
                                 start=True, stop=True)
                nc.scalar.activation(wy_t[h][:, cs(j, 512)], w_ps[:], AF.Copy)
                col = h * NCH + j
                wyc = wy_t[h][:, cs(j, 512)]
                sc = sqp.tile([P, 512], F32, tag="sc", name="sc")
                nc.vector.tensor_scalar(sc[:], wyc, 1.0, 0.0, op0=ALU.mult,
                                        op1=ALU.add,
                                        accum_out=stat_s[:, col:col + 1])
                sq = sqp.tile([P, 512], F32, tag="sq", name="sq")
                nc.vector.scalar_tensor_tensor(sq[:], wyc, 1.0, wyc,
                                               op0=ALU.mult, op1=ALU.mult,
                                               accum_out=stat_q[:, col:col + 1])

        # ---- software-pipelined main loop over flattened (j, k) pairs ----
        TOT = NCH * KPAIR  # 128
        ft_sbs = {}
        y_ps_cur = [None]
        pending_w = []  # (emit_at_iter, j, y_sb)

        for it in range(TOT + LAG):
            # gT convs embedded into the first iterations (2 per iter)
            if it < MCH // 2:
                emit_gt_conv(2 * it)
                emit_gt_conv(2 * it + 1)

            if it < TOT:
                j, k = divmod(it, KPAIR)
                ft_ps = ps_ft.tile([P, 1024], F32, tag="ft", name="ft_ps")
                nc.tensor.matmul(ft_ps[:, 0:512], ph_t[:, cs(2 * k, P)],
                                 th_t[:, cs(j, 512)], start=True, stop=True)
                nc.tensor.matmul(ft_ps[:, 512:1024], ph_t[:, cs(2 * k + 1, P)],
                                 th_t[:, cs(j, 512)], start=True, stop=True)
                ft_sb = ftsb.tile([P, 1024], BF16, tag="ft_sb", name="ft_sb")
                if it % 2 == 0:
                    nc.vector.tensor_copy(ft_sb[:], ft_ps[:])
                else:
                    nc.scalar.activation(ft_sb[:], ft_ps[:], AF.Copy)
                ft_sbs[it] = ft_sb

            while pending_w and pending_w[0][0] <= it:
                _, jw, y_sb_w = pending_w.pop(0)
                emit_w_block(jw, y_sb_w)

            iy = it - LAG
            if 0 <= iy < TOT:
                j2, k2 = divmod(iy, KPAIR)
                if k2 == 0:
                    y_ps_cur[0] = ps_y.tile([P, 512], F32, tag="y", name="y_ps")
                y_ps = y_ps_cur[0]
                ft_sb = ft_sbs.pop(iy)
                nc.tensor.matmul(y_ps[:], gt_t[:, cs(2 * k2, P)],
                                 ft_sb[:, 0:512], start=(k2 == 0), stop=False)
                nc.tensor.matmul(y_ps[:], gt_t[:, cs(2 * k2 + 1, P)],
                                 ft_sb[:, 512:1024], start=False,
                                 stop=(k2 == KPAIR - 1))
                if k2 == KPAIR - 1:
                    y_sb = ysb.tile([P, 512], BF16, tag="y_sb", name="y_sb")
                    nc.vector.tensor_copy(y_sb[:], y_ps[:])
                    pending_w.append((it + 3, j2, y_sb))

        while pending_w:
            _, jw, y_sb_w = pending_w.pop(0)
            emit_w_block(jw, y_sb_w)

        # x32 loaded late (only the tail residual needs it); chunked so it
        # never starves the x16/weight DMAs at startup
        for h in range(2):
            for q in range(4):
                nc.gpsimd.dma_start(x32h[h][:, cs(q, 1024)],
                                    x32[h * P:(h + 1) * P, cs(q, 1024)])

        # ---- BN stats: local reduce, AllReduce, affine params ----
        s4 = pp.tile([P, 4], F32, tag="s4")
        nc.vector.reduce_sum(s4[:, 0:1], stat_s[:, 0:NCH], axis=AX.X)
        nc.vector.reduce_sum(s4[:, 1:2], stat_s[:, NCH:2 * NCH], axis=AX.X)
        nc.vector.reduce_sum(s4[:, 2:3], stat_q[:, 0:NCH], axis=AX.X)
        nc.vector.reduce_sum(s4[:, 3:4], stat_q[:, NCH:2 * NCH], axis=AX.X)
        del stat_s, stat_q
        in_b = dram.tile([P, 4], F32)
        out_b = dram.tile([P * N_CORES, 4], F32)
        nc.sync.dma_start(in_b[:], s4[:])
        nc.gpsimd.collective_compute(
            "AllGather", ALU.bypass,
            replica_groups=[list(range(N_CORES))],
            ins=[in_b.opt()], outs=[out_b.opt()],
        )
        g32 = pp.tile([P, 32], F32, tag="g32")
        nc.sync.dma_start(g32[:].rearrange("p (c r) -> p c r", r=N_CORES),
                          out_b[:].rearrange("(r p) c -> p c r", p=P))
        g4 = pp.tile([P, 4], F32, tag="g4")
        nc.vector.reduce_sum(g4[:], g32[:].rearrange("p (c r) -> p c r",
                                                     r=N_CORES), axis=AX.X)

        inv_cnt = 1.0 / (B * N)
        m4 = pp.tile([P, 4], F32, tag="m4")      # [mn(2) | ms(2)]
        var = pp.tile([P, 2], F32, tag="var")
        tmp = pp.tile([P, 2], F32, tag="tmp")
        sd = pp.tile([P, 2], F32, tag="sd")
        rstd = pp.tile([P, 2], F32, tag="rstd")
        scl = pp.tile([P, 2], F32, tag="scl")
        bia = pp.tile([P, 2], F32, tag="bia")
        nc.vector.tensor_scalar_mul(m4[:], g4[:], inv_cnt)
        mn = m4[:, 0:2]
        nc.vector.tensor_mul(tmp[:], mn, mn)
        nc.vector.tensor_sub(var[:], m4[:, 2:4], tmp[:])
        nc.scalar.activation(sd[:], var[:], AF.Sqrt, bias=eps_t[:, 0:1])
        nc.vector.reciprocal(rstd[:], sd[:])
        nc.vector.tensor_mul(scl[:], rstd[:], gam_t)
        nc.vector.tensor_mul(tmp[:], mn, scl[:])
        nc.vector.tensor_sub(bia[:], bet_t, tmp[:])

        # ---- normalize + residual + store ----
        for idx in range(2 * NCH):
            h, j = divmod(idx, NCH)
            o1 = op.tile([P, 512], F32, tag="o1", name="o1")
            nc.scalar.activation(o1[:], wy_t[h][:, cs(j, 512)], AF.Identity,
                                 bias=bia[:, h:h + 1], scale=scl[:, h:h + 1])
            o2 = op.tile([P, 512], F32, tag="o2", name="o2")
            nc.vector.tensor_add(o2[:], o1[:], x32h[h][:, cs(j, 512)])
            nc.sync.dma_start(out[h * P:(h + 1) * P, cs(j, 512)], o2[:])

    nc.compile()
    return nc


_CACHE = {}


def _get_module():
    if "nc" not in _CACHE:
        _CACHE["nc"] = _build_module()
    return _CACHE["nc"]


def _prep_in_maps(x, g_w, g_b, theta_w, theta_b, phi_w, phi_b, W_w, W_b,
                  bn_gamma, bn_beta):
    bf = ml_dtypes.bfloat16
    f32 = np.float32
    x = np.ascontiguousarray(x, dtype=f32)
    thwT = (theta_w.T / N).astype(bf)
    phwT = phi_w.T.astype(bf)
    gwT = g_w.T.astype(bf)
    WwT = W_w.T.astype(bf)
    wpack = np.concatenate(
        [thwT[0:P], thwT[P:2 * P], phwT[0:P], phwT[P:2 * P],
         gwT[0:P], gwT[P:2 * P], WwT[:, 0:P], WwT[:, P:2 * P]], axis=1)
    bpack = np.concatenate(
        [(theta_b / N).reshape(P, 1).astype(f32),
         phi_b.reshape(P, 1).astype(f32),
         bn_gamma.reshape(2, P).T.astype(f32),
         bn_beta.reshape(2, P).T.astype(f32),
         np.broadcast_to(g_b[None, :].astype(f32), (P, C_OUT))], axis=1)
    shared = {
        "wpack": np.ascontiguousarray(wpack),
        "bpack": np.ascontiguousarray(bpack),
    }
    in_maps = []
    for i in range(N_CORES):
        m = dict(shared)
        m["x32"] = x[i]
        m["x16"] = np.ascontiguousarray(x[i].astype(bf))
        in_maps.append(m)
    return in_maps


def _run(inputs, trace=False, trace_cores=None):
    nc = _get_module()
    in_maps = _prep_in_maps(**inputs)
    res = bass_utils.run_bass_kernel_spmd(
        nc, in_maps, core_ids=list(range(N_CORES)),
        trace=trace, trace_cores=trace_cores,
    )
    out = np.stack([res.results[i]["out"] for i in range(N_CORES)], axis=0)
    return out.astype(np.float32), res


def kernel(**inputs) -> np.ndarray:
    out, _ = _run(inputs, trace=False)
    return out


# revision 12
# speedup vs baseline: 1.0450x; 1.0330x over previous
"""Trainium2 Bass kernel for nn_Attention (non-local-attention block + sync BN).

Computation per batch element b (B=8, C_IN=256, C_OUT=128, N=4096):
    theta = theta_w @ x + theta_b          [128, 4096]
    phi   = phi_w @ x + phi_b              [128, 4096]
    g     = g_w @ x + g_b                  [128, 4096]
    f     = theta^T @ phi / N              [4096, 4096]   (never materialized in DRAM)
    y     = g @ f^T                        [128, 4096]
    w_y   = W_w @ y  (+ W_b, cancels in BN)[256, 4096]
    out   = BN(w_y) * gamma + beta + x     (BN stats over all (B, N) -> AllReduce)

Sharding: data-parallel over batch across 8 NeuronCores (one element per
core); 1x1-conv weights replicated; BN batch stats synced with a tiny
[128,4] fp32 AllReduce.  Compute dtype bf16 (fp32 PSUM accumulation).

Main loop is software-pipelined: the y-matmul for fT pair i is emitted
LAG iterations after the fT matmuls of pair i, so the PSUM->SBUF copy of
fT (split between the Vector and Scalar engines) overlaps with later fT
matmuls and the PE stream stays dense (keeps the HAM clock gate at 2.4GHz).
"""

import contextlib

import numpy as np
import ml_dtypes

import concourse.bass as bass  # noqa: F401  (registers engines)
import concourse.tile as tile
from concourse import bacc, mybir
from concourse import bass_utils

N_CORES = 8
B, C_IN, C_OUT, N = 8, 256, 128, 4096
P = 128
NCH = N // 512    # 8 column chunks of 512
MCH = N // 128    # 32 m-chunks of 128
KPAIR = MCH // 2  # 16 fT pairs per n-chunk
LAG = 3           # y-matmul lag (iterations) behind fT matmuls
BN_EPS = 1e-5

F32 = mybir.dt.float32
BF16 = mybir.dt.bfloat16
AF = mybir.ActivationFunctionType
ALU = mybir.AluOpType
AX = mybir.AxisListType


def _build_module():
    nc = bacc.Bacc("TRN2", target_bir_lowering=False, debug=False,
                   enable_asserts=True, num_devices=N_CORES)

    x32 = nc.dram_tensor("x32", [C_IN, N], F32, kind="ExternalInput").ap()
    x16 = nc.dram_tensor("x16", [C_IN, N], BF16, kind="ExternalInput").ap()
    # wpack columns: thw0 thw1 phw0 phw1 gw0 gw1 WwA WwB (8 x [128,128] bf16)
    wpack = nc.dram_tensor("wpack", [P, 1024], BF16, kind="ExternalInput").ap()
    # bpack columns: thb(1) phb(1) gam(2) bet(2) gbb(128)
    bpack = nc.dram_tensor("bpack", [P, 134], F32, kind="ExternalInput").ap()
    out = nc.dram_tensor("out", [C_IN, N], F32, kind="ExternalOutput").ap()

    with contextlib.ExitStack() as ctx:
        tc = ctx.enter_context(tile.TileContext(nc))
        pp = ctx.enter_context(tc.tile_pool(name="persist", bufs=1))
        ftsb = ctx.enter_context(tc.tile_pool(name="ftsb", bufs=5))
        ysb = ctx.enter_context(tc.tile_pool(name="ysb", bufs=2))
        sqp = ctx.enter_context(tc.tile_pool(name="sqp", bufs=2))
        op = ctx.enter_context(tc.tile_pool(name="outp", bufs=6))
        ps_cv = ctx.enter_context(tc.tile_pool(name="pscv", bufs=2, space="PSUM"))
        ps_ft = ctx.enter_context(tc.tile_pool(name="psft", bufs=2, space="PSUM"))
        ps_y = ctx.enter_context(tc.tile_pool(name="psy", bufs=2, space="PSUM"))
        dram = ctx.enter_context(tc.tile_pool(name="dram", bufs=1, space="DRAM"))

        # ---- persistent SBUF tensors ----
        x16h = [pp.tile([P, N], BF16, tag=f"x16_{h}", name=f"x16_{h}")
                for h in range(2)]
        x32h = [pp.tile([P, N], F32, tag=f"x32_{h}", name=f"x32_{h}")
                for h in range(2)]
        th_t = pp.tile([P, N], BF16, tag="th")
        ph_t = pp.tile([P, N], BF16, tag="ph")
        gt_t = pp.tile([P, N], BF16, tag="gt")       # g^T in 32 [128m x 128c] blocks
        wy_t = [pp.tile([P, N], F32, tag=f"wy{h}", name=f"wy{h}") for h in range(2)]
        stat_s = pp.tile([P, 16], F32, tag="stat_s")  # per-chunk sums
        stat_q = pp.tile([P, 16], F32, tag="stat_q")  # per-chunk sum-of-squares

        wp_t = pp.tile([P, 1024], BF16, tag="wp")
        bp_t = pp.tile([P, 134], F32, tag="bp")
        eps_t = pp.tile([P, 1], F32, tag="eps")
        nc.gpsimd.memset(eps_t[:], BN_EPS)
        warm_t = pp.tile([P, 1], F32, tag="warm")

        def cs(i, w):  # column slice helper
            return slice(i * w, (i + 1) * w)

        # weight DMAs first (small), then x16 chunks so the convs start early,
        # x32 last via SWDGE (only needed for the tail residual)
        nc.sync.dma_start(wp_t[:], wpack[:, :])
        nc.sync.dma_start(bp_t[:], bpack[:, :])
        for q in range(4):
            nc.sync.dma_start(x16h[0][:, cs(q, 1024)], x16[0:P, cs(q, 1024)])
            nc.scalar.dma_start(x16h[1][:, cs(q, 1024)], x16[P:2 * P, cs(q, 1024)])
        thw_t = [wp_t[:, cs(k, P)] for k in range(2)]
        phw_t = [wp_t[:, cs(2 + k, P)] for k in range(2)]
        gw_t = [wp_t[:, cs(4 + k, P)] for k in range(2)]
        Ww_h = [wp_t[:, cs(6 + h, P)] for h in range(2)]
        thb_t = bp_t[:, 0:1]
        phb_t = bp_t[:, 1:2]
        gam_t = bp_t[:, 2:4]
        bet_t = bp_t[:, 4:6]
        gbb_t = bp_t[:, 6:134]

        # dummy tiny AllGather: warms the ncfw path early, overlapped with
        # compute, so the real stats collective at the tail runs at floor cost
        in_d = dram.tile([P, 1], F32)
        out_d = dram.tile([P * N_CORES, 1], F32)
        nc.sync.dma_start(in_d[:], eps_t[:])
        nc.gpsimd.collective_compute(
            "AllGather", ALU.bypass,
            replica_groups=[list(range(N_CORES))],
            ins=[in_d.opt()], outs=[out_d.opt()],
        )

        # ---- phi / theta convs, interleaved per chunk (DMA-paced) ----
        for j in range(NCH):
            for (w_t, b_t, dst) in ((phw_t, phb_t, ph_t), (thw_t, thb_t, th_t)):
                ps = ps_cv.tile([P, 512], F32, tag="cv", name="ps_conv")
                nc.tensor.matmul(ps[:], w_t[0], x16h[0][:, cs(j, 512)],
                                 start=True, stop=False)
                nc.tensor.matmul(ps[:], w_t[1], x16h[1][:, cs(j, 512)],
                                 start=False, stop=True)
                nc.scalar.activation(dst[:, cs(j, 512)], ps[:], AF.Identity,
                                     bias=b_t)

        nc.scalar.activation(warm_t[:], eps_t[:], AF.Sqrt)  # preload ACT table

        def emit_gt_conv(m):
            ps = ps_cv.tile([P, P], F32, tag="cv", name="ps_gt")
            nc.tensor.matmul(ps[:], x16h[0][:, cs(m, P)], gw_t[0],
                             start=True, stop=False)
            nc.tensor.matmul(ps[:], x16h[1][:, cs(m, P)], gw_t[1],
                             start=False, stop=True)
            nc.vector.tensor_tensor(gt_t[:, cs(m, P)], ps[:], gbb_t[:],
                                    op=ALU.add)

        def emit_w_block(j, y_sb):
            for h in range(2):
                w_ps = ps_cv.tile([P, 512], F32, tag="cv", name="ps_w")
                nc.tensor.matmul(w_ps[:], Ww_h[h], y_sb[:],
                                 start=True, stop=True)
                nc.scalar.activation(wy_t[h][:, cs(j, 512)], w_ps[:], AF.Copy)
                col = h * NCH + j
                wyc = wy_t[h][:, cs(j, 512)]
                sc = sqp.tile([P, 512], F32, tag="sc", name="sc")
                nc.vector.tensor_scalar(sc[:], wyc, 1.0, 0.0, op0=ALU.mult,
                                        op1=ALU.add,
                                        accum_out=stat_s[:, col:col + 1])
                sq = sqp.tile([P, 512], F32, tag="sq", name="sq")
                nc.vector.scalar_tensor_tensor(sq[:], wyc, 1.0, wyc,
                                               op0=ALU.mult, op1=ALU.mult,
                                               accum_out=stat_q[:, col:col + 1])

        # ---- software-pipelined main loop over flattened (j, k) pairs ----
        TOT = NCH * KPAIR  # 128
        ft_sbs = {}
        y_ps_cur = [None]
        pending_w = []  # (emit_at_iter, j, y_sb)

        for it in range(TOT + LAG):
            # gT convs embedded into the first iterations (2 per iter)
            if it < MCH // 2:
                emit_gt_conv(2 * it)
                emit_gt_conv(2 * it + 1)

            if it < TOT:
                j, k = divmod(it, KPAIR)
                ft_ps = ps_ft.tile([P, 1024], F32, tag="ft", name="ft_ps")
                nc.tensor.matmul(ft_ps[:, 0:512], ph_t[:, cs(2 * k, P)],
                                 th_t[:, cs(j, 512)], start=True, stop=True)
                nc.tensor.matmul(ft_ps[:, 512:1024], ph_t[:, cs(2 * k + 1, P)],
                                 th_t[:, cs(j, 512)], start=True, stop=True)
                ft_sb = ftsb.tile([P, 1024], BF16, tag="ft_sb", name="ft_sb")
                if it % 2 == 0:
                    nc.vector.tensor_copy(ft_sb[:], ft_ps[:])
                else:
                    nc.scalar.activation(ft_sb[:], ft_ps[:], AF.Copy)
                ft_sbs[it] = ft_sb

            while pending_w and pending_w[0][0] <= it:
                _, jw, y_sb_w = pending_w.pop(0)
                emit_w_block(jw, y_sb_w)

            iy = it - LAG
            if 0 <= iy < TOT:
                j2, k2 = divmod(iy, KPAIR)
                if k2 == 0:
                    y_ps_cur[0] = ps_y.tile([P, 512], F32, tag="y", name="y_ps")
                y_ps = y_ps_cur[0]
                ft_sb = ft_sbs.pop(iy)
                nc.tensor.matmul(y_ps[:], gt_t[:, cs(2 * k2, P)],
                                 ft_sb[:, 0:512], start=(k2 == 0), stop=False)
                nc.tensor.matmul(y_ps[:], gt_t[:, cs(2 * k2 + 1, P)],
                                 ft_sb[:, 512:1024], start=False,
                                 stop=(k2 == KPAIR - 1))
                if k2 == KPAIR - 1:
                    y_sb = ysb.tile([P, 512], BF16, tag="y_sb", name="y_sb")
                    nc.vector.tensor_copy(y_sb[:], y_ps[:])
                    pending_w.append((it + 3, j2, y_sb))

        while pending_w:
            _, jw, y_sb_w = pending_w.pop(0)
            emit_w_block(jw, y_sb_w)

        # x32 loaded late (only the tail residual needs it); chunked so it
        # never starves the x16/weight DMAs at startup
        for h in range(2):
            for q in range(4):
                nc.gpsimd.dma_start(x32h[h][:, cs(q, 1024)],
                                    x32[h * P:(h + 1) * P, cs(q, 1024)])

        # ---- BN stats: local reduce, AllReduce, affine params ----
        s4 = pp.tile([P, 4], F32, tag="s4")
        nc.vector.reduce_sum(s4[:, 0:1], stat_s[:, 0:NCH], axis=AX.X)
        nc.vector.reduce_sum(s4[:, 1:2], stat_s[:, NCH:2 * NCH], axis=AX.X)
        nc.vector.reduce_sum(s4[:, 2:3], stat_q[:, 0:NCH], axis=AX.X)
        nc.vector.reduce_sum(s4[:, 3:4], stat_q[:, NCH:2 * NCH], axis=AX.X)
        del stat_s, stat_q
        in_b = dram.tile([P, 4], F32)
        out_b = dram.tile([P * N_CORES, 4], F32)
        nc.sync.dma_start(in_b[:], s4[:])
        nc.gpsimd.collective_compute(
            "AllGather", ALU.bypass,
            replica_groups=[list(range(N_CORES))],
            ins=[in_b.opt()], outs=[out_b.opt()],
        )
        g32 = pp.tile([P, 32], F32, tag="g32")
        nc.sync.dma_start(g32[:].rearrange("p (c r) -> p c r", r=N_CORES),
                          out_b[:].rearrange("(r p) c -> p c r", p=P))
        g4 = pp.tile([P, 4], F32, tag="g4")
        nc.vector.reduce_sum(g4[:], g32[:].rearrange("p (c r) -> p c r",
                                                     r=N_CORES), axis=AX.X)

        inv_cnt = 1.0 / (B * N)
        m4 = pp.tile([P, 4], F32, tag="m4")      # [mn(2) | ms(2)]
        var = pp.tile([P, 2], F32, tag="var")
        tmp = pp.tile([P, 2], F32, tag="tmp")
        sd = pp.tile([P, 2], F32, tag="sd")
        rstd = pp.tile([P, 2], F32, tag="rstd")
        scl = pp.tile([P, 2], F32, tag="scl")
        bia = pp.tile([P, 2], F32, tag="bia")
        nc.vector.tensor_scalar_mul(m4[:], g4[:], inv_cnt)
        mn = m4[:, 0:2]
        nc.vector.tensor_mul(tmp[:], mn, mn)
        nc.vector.tensor_sub(var[:], m4[:, 2:4], tmp[:])
        nc.scalar.activation(sd[:], var[:], AF.Sqrt, bias=eps_t[:, 0:1])
        nc.vector.reciprocal(rstd[:], sd[:])
        nc.vector.tensor_mul(scl[:], rstd[:], gam_t)
        nc.vector.tensor_mul(tmp[:], mn, scl[:])
        nc.vector.tensor_sub(bia[:], bet_t, tmp[:])

        # ---- normalize + residual + store ----
        for idx in range(2 * NCH):
            h, j = divmod(idx, NCH)
            o1 = op.tile([P, 512], F32, tag="o1", name="o1")
            nc.scalar.activation(o1[:], wy_t[h][:, cs(j, 512)], AF.Identity,
                                 bias=bia[:, h:h + 1], scale=scl[:, h:h + 1])
            o2 = op.tile([P, 512], F32, tag="o2", name="o2")
            nc.vector.tensor_add(o2[:], o1[:], x32h[h][:, cs(j, 512)])
            nc.sync.dma_start(out[h * P:(h + 1) * P, cs(j, 512)], o2[:])

    nc.compile()
    return nc


_CACHE = {}


def _get_module():
    if "nc" not in _CACHE:
        _CACHE["nc"] = _build_module()
    return _CACHE["nc"]


def _prep_in_maps(x, g_w, g_b, theta_w, theta_b, phi_w, phi_b, W_w, W_b,
                  bn_gamma, bn_beta):
    bf = ml_dtypes.bfloat16
    f32 = np.float32
    x = np.ascontiguousarray(x, dtype=f32)
    thwT = (theta_w.T / N).astype(bf)
    phwT = phi_w.T.astype(bf)
    gwT = g_w.T.astype(bf)
    WwT = W_w.T.astype(bf)
    wpack = np.concatenate(
        [thwT[0:P], thwT[P:2 * P], phwT[0:P], phwT[P:2 * P],
         gwT[0:P], gwT[P:2 * P], WwT[:, 0:P], WwT[:, P:2 * P]], axis=1)
    bpack = np.concatenate(
        [(theta_b / N).reshape(P, 1).astype(f32),
         phi_b.reshape(P, 1).astype(f32),
         bn_gamma.reshape(2, P).T.astype(f32),
         bn_beta.reshape(2, P).T.astype(f32),
         np.broadcast_to(g_b[None, :].astype(f32), (P, C_OUT))], axis=1)
    shared = {
        "wpack": np.ascontiguousarray(wpack),
        "bpack": np.ascontiguousarray(bpack),
    }
    in_maps = []
    for i in range(N_CORES):
        m = dict(shared)
        m["x32"] = x[i]
        m["x16"] = np.ascontiguousarray(x[i].astype(bf))
        in_maps.append(m)
    return in_maps


def _run(inputs, trace=False, trace_cores=None):
    nc = _get_module()
    in_maps = _prep_in_maps(**inputs)
    res = bass_utils.run_bass_kernel_spmd(
        nc, in_maps, core_ids=list(range(N_CORES)),
        trace=trace, trace_cores=trace_cores,
    )
    out = np.stack([res.results[i]["out"] for i in range(N_CORES)], axis=0)
    return out.astype(np.float32), res


def kernel(**inputs) -> np.ndarray:
    out, _ = _run(inputs, trace=False)
    return out


# revision 13
# speedup vs baseline: 1.1158x; 1.0677x over previous
"""Trainium2 Bass kernel for nn_Attention (non-local-attention block + sync BN).

Computation per batch element b (B=8, C_IN=256, C_OUT=128, N=4096):
    theta = theta_w @ x + theta_b          [128, 4096]
    phi   = phi_w @ x + phi_b              [128, 4096]
    g     = g_w @ x + g_b                  [128, 4096]
    f     = theta^T @ phi / N              [4096, 4096]   (never materialized in DRAM)
    y     = g @ f^T                        [128, 4096]
    w_y   = W_w @ y  (+ W_b, cancels in BN)[256, 4096]
    out   = BN(w_y) * gamma + beta + x     (BN stats over all (B, N) -> AllReduce)

Sharding: data-parallel over batch across 8 NeuronCores (one element per
core); 1x1-conv weights replicated; BN batch stats synced with a tiny
[128,4] fp32 AllReduce.  Compute dtype bf16 (fp32 PSUM accumulation).

Main loop is software-pipelined: the y-matmul for fT pair i is emitted
LAG iterations after the fT matmuls of pair i, so the PSUM->SBUF copy of
fT (split between the Vector and Scalar engines) overlaps with later fT
matmuls and the PE stream stays dense (keeps the HAM clock gate at 2.4GHz).
"""

import contextlib

import numpy as np
import ml_dtypes

import concourse.bass as bass  # noqa: F401  (registers engines)
import concourse.tile as tile
from concourse import bacc, mybir
from concourse import bass_utils

N_CORES = 8
B, C_IN, C_OUT, N = 8, 256, 128, 4096
P = 128
NCH = N // 512    # 8 column chunks of 512
MCH = N // 128    # 32 m-chunks of 128
KPAIR = MCH // 2  # 16 fT pairs per n-chunk
LAG = 3           # y-matmul lag (iterations) behind fT matmuls
BN_EPS = 1e-5

F32 = mybir.dt.float32
BF16 = mybir.dt.bfloat16
AF = mybir.ActivationFunctionType
ALU = mybir.AluOpType
AX = mybir.AxisListType


def _build_module():
    nc = bacc.Bacc("TRN2", target_bir_lowering=False, debug=False,
                   enable_asserts=True, num_devices=N_CORES)

    x32 = nc.dram_tensor("x32", [C_IN, N], F32, kind="ExternalInput").ap()
    x16 = nc.dram_tensor("x16", [C_IN, N], BF16, kind="ExternalInput").ap()
    # wpack columns: thw0 thw1 phw0 phw1 gw0 gw1 WwA WwB (8 x [128,128] bf16)
    wpack = nc.dram_tensor("wpack", [P, 1024], BF16, kind="ExternalInput").ap()
    # bpack columns: thb(1) phb(1) gam(2) bet(2) gbb(128)
    bpack = nc.dram_tensor("bpack", [P, 134], F32, kind="ExternalInput").ap()
    out = nc.dram_tensor("out", [C_IN, N], F32, kind="ExternalOutput").ap()

    with contextlib.ExitStack() as ctx:
        tc = ctx.enter_context(tile.TileContext(nc))
        pp = ctx.enter_context(tc.tile_pool(name="persist", bufs=1))
        ftsb = ctx.enter_context(tc.tile_pool(name="ftsb", bufs=5))
        ysb = ctx.enter_context(tc.tile_pool(name="ysb", bufs=2))
        sqp = ctx.enter_context(tc.tile_pool(name="sqp", bufs=2))
        op = ctx.enter_context(tc.tile_pool(name="outp", bufs=6))
        ps_cv = ctx.enter_context(tc.tile_pool(name="pscv", bufs=2, space="PSUM"))
        ps_ft = ctx.enter_context(tc.tile_pool(name="psft", bufs=2, space="PSUM"))
        ps_y = ctx.enter_context(tc.tile_pool(name="psy", bufs=2, space="PSUM"))
        dram = ctx.enter_context(tc.tile_pool(name="dram", bufs=1, space="DRAM"))

        # ---- persistent SBUF tensors ----
        x16h = [pp.tile([P, N], BF16, tag=f"x16_{h}", name=f"x16_{h}")
                for h in range(2)]
        x32h = [pp.tile([P, N], F32, tag=f"x32_{h}", name=f"x32_{h}")
                for h in range(2)]
        th_t = pp.tile([P, N], BF16, tag="th")
        ph_t = pp.tile([P, N], BF16, tag="ph")
        gt_t = pp.tile([P, N], BF16, tag="gt")       # g^T in 32 [128m x 128c] blocks
        wy_t = [pp.tile([P, N], F32, tag=f"wy{h}", name=f"wy{h}") for h in range(2)]
        stat_s = pp.tile([P, 16], F32, tag="stat_s")  # per-chunk sums
        stat_q = pp.tile([P, 16], F32, tag="stat_q")  # per-chunk sum-of-squares

        wp_t = pp.tile([P, 1024], BF16, tag="wp")
        bp_t = pp.tile([P, 134], F32, tag="bp")
        eps_t = pp.tile([P, 1], F32, tag="eps")
        nc.gpsimd.memset(eps_t[:], BN_EPS)
        warm_t = pp.tile([P, 1], F32, tag="warm")

        def cs(i, w):  # column slice helper
            return slice(i * w, (i + 1) * w)

        # weight DMAs first (small), then x16 chunks so the convs start early,
        # x32 last via SWDGE (only needed for the tail residual)
        nc.sync.dma_start(wp_t[:], wpack[:, :])
        nc.sync.dma_start(bp_t[:], bpack[:, :])
        for q in range(4):
            nc.sync.dma_start(x16h[0][:, cs(q, 1024)], x16[0:P, cs(q, 1024)])
            nc.scalar.dma_start(x16h[1][:, cs(q, 1024)], x16[P:2 * P, cs(q, 1024)])
        thw_t = [wp_t[:, cs(k, P)] for k in range(2)]
        phw_t = [wp_t[:, cs(2 + k, P)] for k in range(2)]
        gw_t = [wp_t[:, cs(4 + k, P)] for k in range(2)]
        Ww_h = [wp_t[:, cs(6 + h, P)] for h in range(2)]
        thb_t = bp_t[:, 0:1]
        phb_t = bp_t[:, 1:2]
        gam_t = bp_t[:, 2:4]
        bet_t = bp_t[:, 4:6]
        gbb_t = bp_t[:, 6:134]

        # dummy tiny AllGather: warms the ncfw path early, overlapped with
        # compute, so the real stats collective at the tail runs at floor cost
        in_d = dram.tile([P, 1], F32)
        out_d = dram.tile([P * N_CORES, 1], F32)
        nc.sync.dma_start(in_d[:], eps_t[:])
        # x32 on the sync ring AFTER the x16 chunks: ring order means the x16
        # completion semaphores fire before these large transfers start; only
        # the tail residual needs x32
        for h in range(2):
            for q in range(4):
                nc.sync.dma_start(x32h[h][:, cs(q, 1024)],
                                  x32[h * P:(h + 1) * P, cs(q, 1024)])
        nc.gpsimd.collective_compute(
            "AllGather", ALU.bypass,
            replica_groups=[list(range(N_CORES))],
            ins=[in_d.opt()], outs=[out_d.opt()],
        )

        # ---- phi / theta convs, interleaved per chunk (DMA-paced) ----
        for j in range(NCH):
            for (w_t, b_t, dst) in ((phw_t, phb_t, ph_t), (thw_t, thb_t, th_t)):
                ps = ps_cv.tile([P, 512], F32, tag="cv", name="ps_conv")
                nc.tensor.matmul(ps[:], w_t[0], x16h[0][:, cs(j, 512)],
                                 start=True, stop=False)
                nc.tensor.matmul(ps[:], w_t[1], x16h[1][:, cs(j, 512)],
                                 start=False, stop=True)
                nc.scalar.activation(dst[:, cs(j, 512)], ps[:], AF.Identity,
                                     bias=b_t)

        nc.scalar.activation(warm_t[:], eps_t[:], AF.Sqrt)  # preload ACT table

        def emit_gt_conv(m):
            ps = ps_cv.tile([P, P], F32, tag="cv", name="ps_gt")
            nc.tensor.matmul(ps[:], x16h[0][:, cs(m, P)], gw_t[0],
                             start=True, stop=False)
            nc.tensor.matmul(ps[:], x16h[1][:, cs(m, P)], gw_t[1],
                             start=False, stop=True)
            nc.vector.tensor_tensor(gt_t[:, cs(m, P)], ps[:], gbb_t[:],
                                    op=ALU.add)

        def emit_w_block(j, y_sb):
            for h in range(2):
                w_ps = ps_cv.tile([P, 512], F32, tag="cv", name="ps_w")
                nc.tensor.matmul(w_ps[:], Ww_h[h], y_sb[:],
                                 start=True, stop=True)
                nc.scalar.activation(wy_t[h][:, cs(j, 512)], w_ps[:], AF.Copy)
                col = h * NCH + j
                wyc = wy_t[h][:, cs(j, 512)]
                sc = sqp.tile([P, 512], F32, tag="sc", name="sc")
                nc.vector.tensor_scalar(sc[:], wyc, 1.0, 0.0, op0=ALU.mult,
                                        op1=ALU.add,
                                        accum_out=stat_s[:, col:col + 1])
                sq = sqp.tile([P, 512], F32, tag="sq", name="sq")
                nc.vector.scalar_tensor_tensor(sq[:], wyc, 1.0, wyc,
                                               op0=ALU.mult, op1=ALU.mult,
                                               accum_out=stat_q[:, col:col + 1])

        # ---- software-pipelined main loop over flattened (j, k) pairs ----
        TOT = NCH * KPAIR  # 128
        ft_sbs = {}
        y_ps_cur = [None]
        pending_w = []  # (emit_at_iter, j, y_sb)

        for it in range(TOT + LAG):
            # gT convs embedded into the first iterations (2 per iter)
            if it < MCH // 2:
                emit_gt_conv(2 * it)
                emit_gt_conv(2 * it + 1)

            if it < TOT:
                j, k = divmod(it, KPAIR)
                ft_ps = ps_ft.tile([P, 1024], F32, tag="ft", name="ft_ps")
                nc.tensor.matmul(ft_ps[:, 0:512], ph_t[:, cs(2 * k, P)],
                                 th_t[:, cs(j, 512)], start=True, stop=True)
                nc.tensor.matmul(ft_ps[:, 512:1024], ph_t[:, cs(2 * k + 1, P)],
                                 th_t[:, cs(j, 512)], start=True, stop=True)
                ft_sb = ftsb.tile([P, 1024], BF16, tag="ft_sb", name="ft_sb")
                if it % 2 == 0:
                    nc.vector.tensor_copy(ft_sb[:], ft_ps[:])
                else:
                    nc.scalar.activation(ft_sb[:], ft_ps[:], AF.Copy)
                ft_sbs[it] = ft_sb

            while pending_w and pending_w[0][0] <= it:
                _, jw, y_sb_w = pending_w.pop(0)
                emit_w_block(jw, y_sb_w)

            iy = it - LAG
            if 0 <= iy < TOT:
                j2, k2 = divmod(iy, KPAIR)
                if k2 == 0:
                    y_ps_cur[0] = ps_y.tile([P, 512], F32, tag="y", name="y_ps")
                y_ps = y_ps_cur[0]
                ft_sb = ft_sbs.pop(iy)
                nc.tensor.matmul(y_ps[:], gt_t[:, cs(2 * k2, P)],
                                 ft_sb[:, 0:512], start=(k2 == 0), stop=False)
                nc.tensor.matmul(y_ps[:], gt_t[:, cs(2 * k2 + 1, P)],
                                 ft_sb[:, 512:1024], start=False,
                                 stop=(k2 == KPAIR - 1))
                if k2 == KPAIR - 1:
                    y_sb = ysb.tile([P, 512], BF16, tag="y_sb", name="y_sb")
                    nc.vector.tensor_copy(y_sb[:], y_ps[:])
                    pending_w.append((it + 3, j2, y_sb))

        while pending_w:
            _, jw, y_sb_w = pending_w.pop(0)
            emit_w_block(jw, y_sb_w)

        # ---- BN stats: local reduce, AllReduce, affine params ----
        s4 = pp.tile([P, 4], F32, tag="s4")
        nc.vector.reduce_sum(s4[:, 0:1], stat_s[:, 0:NCH], axis=AX.X)
        nc.vector.reduce_sum(s4[:, 1:2], stat_s[:, NCH:2 * NCH], axis=AX.X)
        nc.vector.reduce_sum(s4[:, 2:3], stat_q[:, 0:NCH], axis=AX.X)
        nc.vector.reduce_sum(s4[:, 3:4], stat_q[:, NCH:2 * NCH], axis=AX.X)
        del stat_s, stat_q
        in_b = dram.tile([P, 4], F32)
        out_b = dram.tile([P * N_CORES, 4], F32)
        nc.sync.dma_start(in_b[:], s4[:])
        nc.gpsimd.collective_compute(
            "AllGather", ALU.bypass,
            replica_groups=[list(range(N_CORES))],
            ins=[in_b.opt()], outs=[out_b.opt()],
        )
        g32 = pp.tile([P, 32], F32, tag="g32")
        nc.sync.dma_start(g32[:].rearrange("p (c r) -> p c r", r=N_CORES),
                          out_b[:].rearrange("(r p) c -> p c r", p=P))
        g4 = pp.tile([P, 4], F32, tag="g4")
        nc.vector.reduce_sum(g4[:], g32[:].rearrange("p (c r) -> p c r",
                                                     r=N_CORES), axis=AX.X)

        inv_cnt = 1.0 / (B * N)
        m4 = pp.tile([P, 4], F32, tag="m4")      # [mn(2) | ms(2)]
        var = pp.tile([P, 2], F32, tag="var")
        tmp = pp.tile([P, 2], F32, tag="tmp")
        sd = pp.tile([P, 2], F32, tag="sd")
        rstd = pp.tile([P, 2], F32, tag="rstd")
        scl = pp.tile([P, 2], F32, tag="scl")
        bia = pp.tile([P, 2], F32, tag="bia")
        nc.vector.tensor_scalar_mul(m4[:], g4[:], inv_cnt)
        mn = m4[:, 0:2]
        nc.vector.tensor_mul(tmp[:], mn, mn)
        nc.vector.tensor_sub(var[:], m4[:, 2:4], tmp[:])
        nc.scalar.activation(sd[:], var[:], AF.Sqrt, bias=eps_t[:, 0:1])
        nc.vector.reciprocal(rstd[:], sd[:])
        nc.vector.tensor_mul(scl[:], rstd[:], gam_t)
        nc.vector.tensor_mul(tmp[:], mn, scl[:])
        nc.vector.tensor_sub(bia[:], bet_t, tmp[:])

        # ---- normalize + residual + store ----
        for idx in range(NCH):
            h, j = divmod(idx, NCH // 2)
            o1 = op.tile([P, 1024], F32, tag="o1", name="o1")
            nc.scalar.activation(o1[:], wy_t[h][:, cs(j, 1024)], AF.Identity,
                                 bias=bia[:, h:h + 1], scale=scl[:, h:h + 1])
            o2 = op.tile([P, 1024], F32, tag="o2", name="o2")
            nc.vector.tensor_add(o2[:], o1[:], x32h[h][:, cs(j, 1024)])
            nc.sync.dma_start(out[h * P:(h + 1) * P, cs(j, 1024)], o2[:])

    nc.compile()
    return nc


_CACHE = {}


def _get_module():
    if "nc" not in _CACHE:
        _CACHE["nc"] = _build_module()
    return _CACHE["nc"]


def _prep_in_maps(x, g_w, g_b, theta_w, theta_b, phi_w, phi_b, W_w, W_b,
                  bn_gamma, bn_beta):
    bf = ml_dtypes.bfloat16
    f32 = np.float32
    x = np.ascontiguousarray(x, dtype=f32)
    thwT = (theta_w.T / N).astype(bf)
    phwT = phi_w.T.astype(bf)
    gwT = g_w.T.astype(bf)
    WwT = W_w.T.astype(bf)
    wpack = np.concatenate(
        [thwT[0:P], thwT[P:2 * P], phwT[0:P], phwT[P:2 * P],
         gwT[0:P], gwT[P:2 * P], WwT[:, 0:P], WwT[:, P:2 * P]], axis=1)
    bpack = np.concatenate(
        [(theta_b / N).reshape(P, 1).astype(f32),
         phi_b.reshape(P, 1).astype(f32),
         bn_gamma.reshape(2, P).T.astype(f32),
         bn_beta.reshape(2, P).T.astype(f32),
         np.broadcast_to(g_b[None, :].astype(f32), (P, C_OUT))], axis=1)
    shared = {
        "wpack": np.ascontiguousarray(wpack),
        "bpack": np.ascontiguousarray(bpack),
    }
    in_maps = []
    for i in range(N_CORES):
        m = dict(shared)
        m["x32"] = x[i]
        m["x16"] = np.ascontiguousarray(x[i].astype(bf))
        in_maps.append(m)
    return in_maps


def _run(inputs, trace=False, trace_cores=None):
    nc = _get_module()
    in_maps = _prep_in_maps(**inputs)
    res = bass_utils.run_bass_kernel_spmd(
        nc, in_maps, core_ids=list(range(N_CORES)),
        trace=trace, trace_cores=trace_cores,
    )
    out = np.stack([res.results[i]["out"] for i in range(N_CORES)], axis=0)
    return out.astype(np.float32), res


def kernel(**inputs) -> np.ndarray:
    out, _ = _run(inputs, trace=False)
    return out
